# revision 18
# baseline (speedup 1.0000x reference)
"""Trainium2 Bass kernel for nn_MoEFFNBlock (B=2,S=2048,D=1024,H=2048,E=8,K=2).

Strategy (expert-parallel, 8 cores):
  host: fp32 router (softmax+top2, validated to match the jax reference
        selection), gather tokens per expert, fold the normalized top-k
        combine weight into the expert output on-device.
  core e: expert-e SwiGLU FFN over its <=C gathered tokens with bf16
        matmuls (full PE rate, ~5e-3 rel err), plus a 256-wide H-shard of
        the shared expert over all T tokens.
  host: scatter-add per-expert outputs + sum the 8 shared-expert partials.

All matmul operands are bf16 (half the HBM traffic of fp32, same PE
rate); PSUM accumulation stays fp32. The per-token combine weight is
applied AFTER the down-projection (it commutes with the matmul), so the
h-stage writes silu(g)*u directly. DMA triggers cost ~0.6us each on the
in-order Sync sequencer, so transfers are batched: gate+up weights ship
as one interleaved tensor per H-tile, and outputs use partition-major
dram layouts so each expert-output row block / shared-output token chunk
is a single descriptor-matched dma_start.
"""

import json
import math

import numpy as np

_B, _S, _D, _H, _E = 2, 2048, 1024, 2048, 8
_T = _B * _S
_P = 128
_NC = 8
_HSH = _H // _NC  # shared-expert H columns per core
_DK = _D // _P  # 8 contraction tiles over D
_HT = _H // _P  # 16 tiles over H
_SK = _HSH // _P  # 2 contraction tiles over the H-shard
_TC = 512  # shared-expert token chunk
_NWARM = 20  # PE p-state warmup matmuls (256-wide)

_TPB_ENGINES = {"PE", "Activation", "DVE", "Pool", "SP"}


def _split_waits(bir_bytes: bytes) -> bytes:
    """walrus in this container accepts only one sync-wait per TPB
    instruction; Tile's tail drain carries several. Hoist extras onto
    NoOps that run just before the instruction on the same engine."""
    m = json.loads(bir_bytes)
    ctr = 0
    for f in m["functions"]:
        blocks = f["blocks"]
        items = blocks.items() if isinstance(blocks, dict) else enumerate(blocks)
        for _bname, bb in items:
            new_insts = []
            for inst in bb["instructions"]:
                si = inst.get("sync_info") or {}
                ow = si.get("on_wait") or []
                if len(ow) > 1 and inst.get("engine") in _TPB_ENGINES:
                    for w in ow[:-1]:
                        ctr += 1
                        nop = {
                            "name": f"I-waitsplit-{ctr}",
                            "engine": inst["engine"],
                            "opcode": "NoOp",
                            "ins": [],
                            "outs": [],
                            "sync_info": {"on_wait": [w], "on_update": []},
                        }
                        if "debug" in inst:
                            nop["debug"] = inst["debug"]
                        new_insts.append(nop)
                    si["on_wait"] = [ow[-1]]
                new_insts.append(inst)
            bb["instructions"] = new_insts
    return json.dumps(m).encode()


def _chunks(C):
    """Column chunks of near-equal width (<=512, >=256 when possible):
    matmuls narrower than ~300 pay a measurable LDWEIGHTS-shadow penalty
    (+8.6ns avg at 256 vs +3.7 at 303), so equalize instead of taking
    greedy 512s. Ascending so the first chunk's input DMA (the critical
    path at kernel start) is smallest. C >= 512."""
    n = -(-C // 512)
    base, extra = divmod(C, n)
    widths = sorted([base + (1 if i < extra else 0) for i in range(n)])
    ccs, o = [], 0
    for w in widths:
        assert w <= 512 and (w >= 256 or n == 1)
        ccs.append((o, w))
        o += w
    assert o == C
    return ccs


def _build(C):
    import concourse.bass as bass
    import concourse.mybir as mybir
    import concourse.tile as tile

    f32 = mybir.dt.float32
    bf16 = mybir.dt.bfloat16
    Silu = mybir.ActivationFunctionType.Silu
    mult = mybir.AluOpType.mult

    nc = bass.Bass(trn_type="TRN2")
    xe = nc.dram_tensor("xe", [_P, _DK, C], bf16, kind="ExternalInput")
    cw = nc.dram_tensor("cw", [_P, C], f32, kind="ExternalInput")
    wgu = nc.dram_tensor("wgu", [_HT, _P, 2, _DK, _P], bf16, kind="ExternalInput")
    wd = nc.dram_tensor("wd", [_DK, _P, _HT, _P], bf16, kind="ExternalInput")
    xt = nc.dram_tensor("xt", [_P, _DK, _T], bf16, kind="ExternalInput")
    sg = nc.dram_tensor("sg", [_P, _DK, _HSH], bf16, kind="ExternalInput")
    su = nc.dram_tensor("su", [_P, _DK, _HSH], bf16, kind="ExternalInput")
    sd = nc.dram_tensor("sd", [_P, _SK, _D], bf16, kind="ExternalInput")
    rout = nc.dram_tensor("rout", [_P, _DK, C], bf16, kind="ExternalOutput")
    shout = nc.dram_tensor("shout", [_P, _DK, _T], bf16, kind="ExternalOutput")

    ccs = _chunks(C)

    with tile.TileContext(nc) as tc:
        with (
            tc.tile_pool(name="tmp", bufs=2) as tmp,
            tc.tile_pool(name="ps", bufs=2, space="PSUM") as psp,
            tc.tile_pool(name="bigS", bufs=1) as bigS,
            tc.tile_pool(name="cwg", bufs=1) as cwg,
            tc.tile_pool(name="strDW", bufs=2) as strDW,
        ):
            # PE warmup: dummy matmuls so HAM un-throttles while the
            # initial DMAs are in flight.
            wtile32 = cwg.tile([_P, 256], f32, name="wtile32")
            nc.vector.memset(wtile32[:], 0.0)
            wtile = cwg.tile([_P, 256], bf16, name="wtile")
            nc.vector.tensor_copy(wtile[:], wtile32[:])
            # The warmup borrows one of the d-stage 'out' PSUM buffers
            # (first real use is ~100us later) so all 8 banks stay
            # available for the h1/h2/out rotation.
            wps = psp.tile([_P, 512], f32, tag="out", name="ops", bufs=4)[:, :256]
            for i in range(_NWARM):
                nc.tensor.matmul(
                    wps,
                    wtile[:, :_P],
                    wtile[:],
                    start=(i == 0),
                    stop=(i == _NWARM - 1),
                )

            cw_sb = cwg.tile([_P, C], f32, name="cw_sb")
            g_sb = cwg.tile([_P, _HT, C], bf16, name="g_sb")
            sg_sb = bigS.tile([_P, _DK, _HSH], bf16, name="sg_sb")
            su_sb = bigS.tile([_P, _DK, _HSH], bf16, name="su_sb")
            sd_sb = bigS.tile([_P, _SK, _D], bf16, name="sd_sb")

            # ---------- phase R / h-stage: g = silu(Wg x) * (Wu x) ------
            with (
                tc.tile_pool(name="poolXE", bufs=1) as poolXE,
                tc.tile_pool(name="strGU", bufs=3) as strGU,
                tc.tile_pool(name="strS", bufs=3) as strS,
                tc.tile_pool(name="strO", bufs=2) as strO,
            ):
                # First-needed data first: the (smallest) xe chunk 0, then
                # the ht=0 gate half, the ht=0 up half, remaining xe
                # chunks, ht=1 weights. Shared-expert weights and cw are
                # deferred into the ht loop (cw is first used by the
                # d-stage combine multiply).
                xe_sb = poolXE.tile([_P, _DK, C], bf16, name="xe_sb")
                c0_, cn_ = ccs[0]
                nc.sync.dma_start(
                    xe_sb[:, :, c0_ : c0_ + cn_], xe.ap()[:, :, c0_ : c0_ + cn_]
                )
                wgu_tiles = []
                w0 = strGU.tile([_P, 2, _DK, _P], bf16, tag="wgu", name="wgu_t")
                nc.sync.dma_start(w0[:, 0], wgu.ap()[0][:, 0])
                nc.sync.dma_start(w0[:, 1], wgu.ap()[0][:, 1])
                wgu_tiles.append(w0)
                for c0_, cn_ in ccs[1:]:
                    nc.sync.dma_start(
                        xe_sb[:, :, c0_ : c0_ + cn_], xe.ap()[:, :, c0_ : c0_ + cn_]
                    )
                w1 = strGU.tile([_P, 2, _DK, _P], bf16, tag="wgu", name="wgu_t")
                nc.sync.dma_start(w1[:], wgu.ap()[1])
                wgu_tiles.append(w1)

                for ht in range(_HT):
                    if ht < 2:
                        wgu_t = wgu_tiles[ht]
                    else:
                        wgu_t = strGU.tile(
                            [_P, 2, _DK, _P], bf16, tag="wgu", name="wgu_t"
                        )
                        nc.sync.dma_start(wgu_t[:], wgu.ap()[ht])
                    if ht == 3:
                        nc.sync.dma_start(sg_sb[:], sg.ap())
                        nc.sync.dma_start(su_sb[:], su.ap())
                    elif ht == 5:
                        nc.sync.dma_start(sd_sb[:], sd.ap())
                    elif ht == 7:
                        nc.sync.dma_start(cw_sb[:], cw.ap())
                    for c0, cn in ccs:
                        h1 = psp.tile([_P, 512], f32, tag="h1", name="h1ps")[:, :cn]
                        for k in range(_DK):
                            nc.tensor.matmul(
                                h1,
                                wgu_t[:, 0, k],
                                xe_sb[:, k, c0 : c0 + cn],
                                start=(k == 0),
                                stop=(k == _DK - 1),
                            )
                        h2 = psp.tile([_P, 512], f32, tag="h2", name="h2ps", bufs=2)[:, :cn]
                        for k in range(_DK):
                            nc.tensor.matmul(
                                h2,
                                wgu_t[:, 1, k],
                                xe_sb[:, k, c0 : c0 + cn],
                                start=(k == 0),
                                stop=(k == _DK - 1),
                            )
                        sl = tmp.tile([_P, 512], f32, tag="sl", name="sl_sb")[
                            :, :cn
                        ]
                        nc.scalar.activation(sl, h1, Silu)
                        nc.vector.tensor_tensor(
                            g_sb[:, ht, c0 : c0 + cn], sl, h2, mult
                        )

                # ---------- phase R / d-stage + phase S ------------------
                tchunks = [(i * _TC, _TC) for i in range(_T // _TC)]

                # Prefetch the first two shared-expert token chunks now so
                # the in-order Sync queue can't head-of-line block them
                # behind the d-stage output DMAs.
                xt_pre = []
                for t0, tc in tchunks[:2]:
                    xt_sb = strS.tile([_P, _DK, _TC], bf16, tag="xt", name="xt_sb")
                    nc.sync.dma_start(
                        xt_sb[:, :, :tc], xt.ap()[:, :, t0 : t0 + tc]
                    )
                    xt_pre.append(xt_sb)

                for dt_i in range(_DK):
                    wd_t = strDW.tile([_P, _HT, _P], bf16, tag="wd", name="wd_t")
                    nc.sync.dma_start(wd_t[:], wd.ap()[dt_i])
                    ro = strO.tile([_P, C], bf16, tag="ro", name="ro_sb")
                    for c0, cn in ccs:
                        ops = psp.tile(
                            [_P, 512], f32, tag="out", name="ops", bufs=4
                        )[:, :cn]
                        for k in range(_HT):
                            nc.tensor.matmul(
                                ops,
                                wd_t[:, k],
                                g_sb[:, k, c0 : c0 + cn],
                                start=(k == 0),
                                stop=(k == _HT - 1),
                            )
                        nc.vector.tensor_tensor(
                            ro[:, c0 : c0 + cn], ops, cw_sb[:, c0 : c0 + cn], mult
                        )
                    nc.sync.dma_start(rout.ap()[:, dt_i], ro)

                # phase S, software-pipelined: d-stage trails one chunk.
                def s_hstage(ci, t0, tc):
                    if ci < len(xt_pre):
                        xt_sb = xt_pre[ci]
                    else:
                        xt_sb = strS.tile(
                            [_P, _DK, _TC], bf16, tag="xt", name="xt_sb"
                        )
                        nc.sync.dma_start(
                            xt_sb[:, :, :tc], xt.ap()[:, :, t0 : t0 + tc]
                        )
                    gs = strS.tile([_P, _SK, _TC], bf16, tag="gs", name="gs_sb")
                    for hs in range(_SK):
                        h1 = psp.tile([_P, 512], f32, tag="h1", name="h1ps")[:, :tc]
                        for k in range(_DK):
                            nc.tensor.matmul(
                                h1,
                                sg_sb[:, k, hs * _P : (hs + 1) * _P],
                                xt_sb[:, k, :tc],
                                start=(k == 0),
                                stop=(k == _DK - 1),
                            )
                        h2 = psp.tile([_P, 512], f32, tag="h2", name="h2ps", bufs=2)[
                            :, :tc
                        ]
                        for k in range(_DK):
                            nc.tensor.matmul(
                                h2,
                                su_sb[:, k, hs * _P : (hs + 1) * _P],
                                xt_sb[:, k, :tc],
                                start=(k == 0),
                                stop=(k == _DK - 1),
                            )
                        sl = tmp.tile([_P, 512], f32, tag="sl", name="sl_sb")[:, :tc]
                        nc.scalar.activation(sl, h1, Silu)
                        nc.vector.tensor_tensor(gs[:, hs, :tc], sl, h2, mult)
                    return gs

                def s_dstage(t0, tc, gs, last=False):
                    # Ship the outputs in dt-halves (quarters on the very
                    # last chunk) so the final exposed DMA after the last
                    # matmul is small; earlier pieces overlap compute.
                    cuts = (4, 6, 8) if last else (4, 8)
                    so = strO.tile([_P, _DK, _TC], bf16, tag="so", name="so_sb")
                    lo = 0
                    for dt_i in range(_DK):
                        ops = psp.tile(
                            [_P, 512], f32, tag="out", name="ops", bufs=4
                        )[:, :tc]
                        for k in range(_SK):
                            nc.tensor.matmul(
                                ops,
                                sd_sb[:, k, dt_i * _P : (dt_i + 1) * _P],
                                gs[:, k, :tc],
                                start=(k == 0),
                                stop=(k == _SK - 1),
                            )
                        if dt_i % 2:
                            nc.scalar.copy(so[:, dt_i, :tc], ops)
                        else:
                            nc.vector.tensor_copy(so[:, dt_i, :tc], ops)
                        if dt_i + 1 in cuts:
                            nc.sync.dma_start(
                                shout.ap()[:, lo : dt_i + 1, t0 : t0 + tc],
                                so[:, lo : dt_i + 1, :tc],
                            )
                            lo = dt_i + 1

                prev = None
                for ci, (t0, tc) in enumerate(tchunks):
                    gs = s_hstage(ci, t0, tc)
                    if prev is not None:
                        s_dstage(prev[0], prev[1], prev[2])
                    prev = (t0, tc, gs)
                s_dstage(prev[0], prev[1], prev[2], last=True)

    orig = nc.to_json_bytes
    nc.to_json_bytes = lambda: _split_waits(orig())
    return nc


def _route(xf, w_router):
    """fp32 router matching the jax reference: softmax over logits, top-2
    (selection identical to jax.lax.top_k for non-tied logits), weights
    renormalized over the selected pair."""
    logits = xf @ w_router.T.astype(np.float32)
    m = logits.max(-1, keepdims=True)
    p = np.exp(logits - m)
    p /= p.sum(-1, keepdims=True)
    i1 = p.argmax(-1)
    p2 = p.copy()
    p2[np.arange(p.shape[0]), i1] = -1.0
    i2 = p2.argmax(-1)
    w1 = p[np.arange(p.shape[0]), i1]
    w2 = p[np.arange(p.shape[0]), i2]
    s = w1 + w2
    return i1, i2, (w1 / s).astype(np.float32), (w2 / s).astype(np.float32)


def _tile_kxm(a2d, kouter):
    """[K, M] -> [128, K//128, M] with partition dim first."""
    k, mdim = a2d.shape
    assert k == kouter * _P
    return np.ascontiguousarray(a2d.reshape(kouter, _P, mdim).transpose(1, 0, 2))


def _prepare(inputs):
    import ml_dtypes

    bf16 = ml_dtypes.bfloat16

    x = np.asarray(inputs["x"], dtype=np.float32)
    w_router = np.asarray(inputs["w_router"], dtype=np.float32)
    Wg = np.asarray(inputs["Wg"], dtype=np.float32)
    Wu = np.asarray(inputs["Wu"], dtype=np.float32)
    Wd = np.asarray(inputs["Wd"], dtype=np.float32)
    sg = np.asarray(inputs["sg"], dtype=np.float32)
    su = np.asarray(inputs["su"], dtype=np.float32)
    sd = np.asarray(inputs["sd"], dtype=np.float32)

    xf = np.ascontiguousarray(x.reshape(_T, _D))
    i1, i2, w1, w2 = _route(xf, w_router)

    idxs, cws = [], []
    for e in range(_E):
        sel = (i1 == e) | (i2 == e)
        idx = np.nonzero(sel)[0]
        cwv = np.where(i1[idx] == e, w1[idx], w2[idx]).astype(np.float32)
        idxs.append(idx)
        cws.append(cwv)
    cmax = max(len(i) for i in idxs)
    C = max(512, cmax)

    xt_h = _tile_kxm(np.ascontiguousarray(xf.T), _DK).astype(bf16)  # [P, DK, T]

    in_maps = []
    for e in range(_E):
        idx, cwv = idxs[e], cws[e]
        n = len(idx)
        xe_h = np.zeros((_P, _DK, C), bf16)
        if n:
            xe_h[:, :, :n] = _tile_kxm(np.ascontiguousarray(xf[idx].T), _DK).astype(
                bf16
            )
        cw_h = np.zeros((_P, C), np.float32)
        cw_h[:, :n] = cwv[None, :]

        wgT = np.ascontiguousarray(Wg[e].T)  # [D, H]
        wg_h = np.ascontiguousarray(
            wgT.reshape(_DK, _P, _HT, _P).transpose(2, 1, 0, 3)
        ).astype(bf16)
        wuT = np.ascontiguousarray(Wu[e].T)
        wu_h = np.ascontiguousarray(
            wuT.reshape(_DK, _P, _HT, _P).transpose(2, 1, 0, 3)
        ).astype(bf16)
        wgu_h = np.ascontiguousarray(
            np.stack([wg_h, wu_h], axis=2)
        )  # [HT, P, 2, DK, P]
        wdT = np.ascontiguousarray(Wd[e].T)  # [H, D]
        wd_h = np.ascontiguousarray(
            wdT.reshape(_HT, _P, _DK, _P).transpose(2, 1, 0, 3)
        ).astype(bf16)

        hs = slice(e * _HSH, (e + 1) * _HSH)
        sg_h = _tile_kxm(np.ascontiguousarray(sg[hs].T), _DK).astype(bf16)
        su_h = _tile_kxm(np.ascontiguousarray(su[hs].T), _DK).astype(bf16)
        sd_h = _tile_kxm(np.ascontiguousarray(sd[:, hs].T), _SK).astype(bf16)

        in_maps.append(
            {
                "xe": xe_h,
                "cw": cw_h,
                "wgu": wgu_h,
                "wd": wd_h,
                "xt": xt_h,
                "sg": sg_h,
                "su": su_h,
                "sd": sd_h,
            }
        )
    return in_maps, idxs, C


def _combine(results, idxs):
    out = np.zeros((_D, _T), np.float32)
    for e in range(_E):
        # shout/rout are [P, DK, cols]; D index is dk*128 + p.
        sh = results[e]["shout"].astype(np.float32).transpose(1, 0, 2).reshape(_D, _T)
        out += sh
        idx = idxs[e]
        if len(idx):
            ro = (
                results[e]["rout"].astype(np.float32).transpose(1, 0, 2).reshape(_D, -1)
            )
            out[:, idx] += ro[:, : len(idx)]
    return np.ascontiguousarray(out.T).reshape(_B, _S, _D).astype(np.float32)


def kernel(**inputs):
    from concourse import bass_utils

    in_maps, idxs, C = _prepare(inputs)
    nc = _build(C)
    res = bass_utils.run_bass_kernel_spmd(nc, in_maps, core_ids=list(range(_NC)))
    return _combine(res.results, idxs)
